# revision 7
# baseline (speedup 1.0000x reference)
"""Multi-head cross-attention on 8 Trainium2 NeuronCores.

Problem shapes (hardcoded): B=4, Ld=1024, Le=2048, d_model=1024, 8 heads x 128.
Sharding: core c handles batch b=c//2 and head-group g=c%2 (4 heads each).
Each core computes q/k/v projections for its heads, attention, and a partial
output projection over its heads' value dims; the host sums the two partial
outputs per batch and adds the (bias-folded) output bias.

Key scheduling/engine decisions:
- Projection inputs (enc, x, wq, wk, wv) ship as bf16 in chunk-major host
  layout ([128, chunks*cols]), so each tensor is one large contiguous DMA;
  only the cold-start K-proj inputs stream per chunk.
- Projections run d-major across all 8 PSUM banks; bank copy-backs alternate
  DVE/Act so the next phase's bank-0 chain never waits.
- Softmax denominators never touch the PE: exp'd score chunks (bf16)
  accumulate on DVE (2x mode), then one gpsimd partition_all_reduce per
  (q-half, head) produces the broadcast denominator, reciprocal on DVE.
- exp processes two key-chunks per Act instruction ([128,1024] spanning two
  PSUM banks) to keep Act throughput above the PE's score+AV rate.
- b_v is folded into the host-side output bias (b_eff = b_o + W_o @ b_v),
  removing all bias work from the attention loop.
- Attention is software-pipelined: one filler matmul per inner step keeps
  the PE busy while Act computes exp. Fillers for q-half 0 are the second
  half of the Q projection; fillers for q-half 1 are q-half 0's output
  projection. The PE is warmed with dummy matmuls on a memset tile so the
  p-state ramp completes before real work arrives.
"""

import math
import sys

import numpy as np

for _p in ("/opt/trn_rl_repo", "/root/.axon_site/_ro/trn_rl_repo"):
    if _p not in sys.path:
        sys.path.append(_p)

B = 4
LQ = 1024
LK = 2048
D = 1024
H = 8
DH = 128
P = 128
HPC = 4          # heads per core
OQ = HPC * DH    # 512 projected dims per core
NQ = 512         # matmul moving free dim
KC = D // P      # 8 contraction chunks for projections
LKC = LK // P    # 16 key chunks
HLK = LK // 2    # 1024, one lk-half of the encoder
N_CORES = 8

_BUILT = {}


def _build(masked):
    import concourse.bass as bass  # noqa: F401
    import concourse.tile as tile
    import concourse.mybir as mybir
    from concourse import bacc
    from concourse import bass_isa

    f32 = mybir.dt.float32
    f32r = mybir.dt.float32r
    bf16 = mybir.dt.bfloat16
    Exp = mybir.ActivationFunctionType.Exp

    nc = bacc.Bacc("TRN2", target_bir_lowering=False, debug=False,
                   num_devices=N_CORES)

    # chunk-major DRAM layouts: [...][p, d*cols + c] = chunk d, row p, col c
    xr = nc.dram_tensor("xr", [P, 2 * KC * NQ], bf16, kind="ExternalInput").ap()
    e0r = nc.dram_tensor("e0r", [P, KC * HLK], bf16, kind="ExternalInput").ap()
    e1r = nc.dram_tensor("e1r", [P, KC * HLK], bf16, kind="ExternalInput").ap()
    wqr = nc.dram_tensor("wqr", [P, KC * OQ], bf16, kind="ExternalInput").ap()
    wkr = nc.dram_tensor("wkr", [P, KC * OQ], bf16, kind="ExternalInput").ap()
    wvr = nc.dram_tensor("wvr", [P, KC * OQ], bf16, kind="ExternalInput").ap()
    wor = nc.dram_tensor("wor", [P, HPC * D], f32r, kind="ExternalInput").ap()
    bq_d = nc.dram_tensor("bq", [P, HPC], f32, kind="ExternalInput").ap()
    bk_d = nc.dram_tensor("bk", [P, HPC], f32, kind="ExternalInput").ap()
    if masked:
        maskT = nc.dram_tensor("maskT", [LK, LQ], f32, kind="ExternalInput").ap()
    out_d = nc.dram_tensor("out", [LQ, D], f32, kind="ExternalOutput").ap()

    with tile.TileContext(nc) as tc:
        with tc.tile_pool(name="persist", bufs=1) as persist:
            kT = [persist.tile([P, LK], f32r, name=f"kT{h}") for h in range(HPC)]
            qT = [persist.tile([P, LQ], f32r, name=f"qT{h}") for h in range(HPC)]
            vch = [persist.tile([P, OQ], bf16, name=f"v{j}") for j in range(LKC)]
            wq_all = persist.tile([P, KC * OQ], bf16, name="wq")
            wo_all = persist.tile([P, HPC * D], f32r, name="wo")
            x_all = persist.tile([P, 2 * KC * NQ], bf16, name="x")
            bq_sb = persist.tile([P, HPC], f32, name="bq")
            bk_sb = persist.tile([P, HPC], f32, name="bk")
            warm = persist.tile([P, 256], bf16, name="warm")

            wqc = [wq_all[:, d * OQ:(d + 1) * OQ] for d in range(KC)]
            woch = [wo_all[:, h * D:(h + 1) * D] for h in range(HPC)]
            xg0 = [x_all[:, d * NQ:(d + 1) * NQ] for d in range(KC)]
            xg1 = [x_all[:, (KC + d) * NQ:(KC + d + 1) * NQ] for d in range(KC)]

            with (
                tc.tile_pool(name="acc", bufs=1, space="PSUM") as accp,
                tc.tile_pool(name="wk", bufs=1) as wkp,
                tc.tile_pool(name="wv", bufs=1) as wvp,
                tc.tile_pool(name="e0p", bufs=1) as e0p,
                tc.tile_pool(name="e1p", bufs=1) as e1p,
            ):
                banks = [accp.tile([P, NQ], f32, name=f"bank{t}")
                         for t in range(8)]
                wk_all = wkp.tile([P, KC * OQ], bf16, name="wk")
                wv_all = wvp.tile([P, KC * OQ], bf16, name="wv")
                e0_all = e0p.tile([P, KC * HLK], bf16, name="e0")
                e1_all = e1p.tile([P, KC * HLK], bf16, name="e1")
                wkc = [wk_all[:, d * OQ:(d + 1) * OQ] for d in range(KC)]
                wvc = [wv_all[:, d * OQ:(d + 1) * OQ] for d in range(KC)]
                e0 = [e0_all[:, d * HLK:(d + 1) * HLK] for d in range(KC)]
                e1 = [e1_all[:, d * HLK:(d + 1) * HLK] for d in range(KC)]

                # --- PE warm-up: memset a tile (no DMA) and run dummy
                # matmuls so the p-state ramp happens before real work.
                nc.vector.memset(warm[:], 1.0)
                for _ in range(14):
                    nc.tensor.matmul(banks[0][0:16, 0:256], warm[:, 0:16],
                                     warm[:], start=True, stop=True)

                # --- DMA program: cold-start K inputs per chunk, the rest
                # as single contiguous transfers in need-order.
                for d in range(KC):
                    nc.sync.dma_start(wkc[d], wkr[:, d * OQ:(d + 1) * OQ])
                    nc.sync.dma_start(e0[d], e0r[:, d * HLK:(d + 1) * HLK])
                nc.sync.dma_start(bq_sb[:], bq_d[:])
                nc.sync.dma_start(bk_sb[:], bk_d[:])
                nc.sync.dma_start(wv_all[:], wvr[:])
                nc.sync.dma_start(e1_all[:], e1r[:])
                nc.sync.dma_start(wq_all[:], wqr[:])
                nc.sync.dma_start(x_all[:], xr[:])
                nc.sync.dma_start(wo_all[:], wor[:])

                def kproj_half(e, lh):
                    for d in range(KC):
                        for g in range(2):
                            for h in range(HPC):
                                nc.tensor.matmul(
                                    banks[g * 4 + h][:],
                                    wkc[d][:, h * DH:(h + 1) * DH],
                                    e[d][:, g * NQ:(g + 1) * NQ],
                                    start=(d == 0), stop=(d == KC - 1))
                    for i in range(8):
                        g, h = i // 4, i % 4
                        dst = kT[h][:, lh * HLK + g * NQ:lh * HLK + (g + 1) * NQ]
                        if i % 2 == 0:
                            nc.vector.tensor_scalar_add(
                                dst, banks[i][:], bk_sb[:, h:h + 1])
                        else:
                            nc.scalar.add(dst, banks[i][:], bk_sb[:, h:h + 1])

                def vproj_half(e, lh):
                    for d in range(KC):
                        for j8 in range(8):
                            nc.tensor.matmul(
                                banks[j8][:],
                                e[d][:, j8 * P:(j8 + 1) * P],
                                wvc[d],
                                start=(d == 0), stop=(d == KC - 1))
                    for j8 in range(8):
                        dst = vch[lh * 8 + j8][:]
                        if j8 % 2 == 0:
                            nc.vector.tensor_copy(dst, banks[j8][:])
                        else:
                            nc.scalar.copy(dst, banks[j8][:])

                kproj_half(e0, 0)
                vproj_half(e0, 0)
                kproj_half(e1, 1)
                vproj_half(e1, 1)

                # Q proj, query half 0, in two 2-head subphases so the
                # head-0/1 copy-backs finish before attention needs qT.
                for sub in range(2):
                    hs = (0, 1) if sub == 0 else (2, 3)
                    for d in range(KC):
                        for h in hs:
                            nc.tensor.matmul(
                                banks[h][:],
                                wqc[d][:, h * DH:(h + 1) * DH],
                                xg0[d],
                                start=(d == 0), stop=(d == KC - 1))
                    for h in hs:
                        nc.scalar.add(qT[h][:, 0:NQ], banks[h][:],
                                      bq_sb[:, h:h + 1])

            # ---- Attention, software-pipelined with filler matmuls.
            with (
                tc.tile_pool(name="pTp", bufs=6) as pTp,
                tc.tile_pool(name="dnp", bufs=2) as dnp,
                tc.tile_pool(name="maskp", bufs=16 if masked else 1) as maskp,
                tc.tile_pool(name="osb", bufs=4) as osb,
                tc.tile_pool(name="att", bufs=1) as attp,
                tc.tile_pool(name="pss", bufs=2, space="PSUM") as pss,
                tc.tile_pool(name="psa", bufs=2, space="PSUM") as psa,
                tc.tile_pool(name="psx", bufs=2, space="PSUM") as psx,
            ):
                valsT = [attp.tile([P, LQ], f32r, name=f"valsT{h}")
                         for h in range(HPC)]
                partial = [attp.tile([P, NQ], f32, name=f"prt{c}")
                           for c in range(8)]

                def attn_q2(q2, slot_fillers):
                    """h-loops for one query half. slot_fillers: 32 lists of
                    callables (slot = h*8+g), each emitting one PE matmul
                    (+ its own non-PE follow-ups)."""

                    def fill(slot):
                        for f in slot_fillers[slot]:
                            f()

                    if masked:
                        mch = []
                        for j in range(LKC):
                            mt = maskp.tile([P, NQ], f32, name=f"m{j}")
                            nc.sync.dma_start(
                                mt[:], maskT[j * P:(j + 1) * P,
                                             q2 * NQ:(q2 + 1) * NQ])
                            mch.append(mt)

                    for h in range(HPC):
                        qs = qT[h][:, q2 * NQ:(q2 + 1) * NQ]
                        ps_v = psa.tile([P, NQ], f32, name="ps_v")
                        pT = [None] * 8
                        acc = None

                        def spair(g):
                            t = pss.tile([P, 2 * NQ], f32, name="ps_s")
                            for jj in range(2):
                                j = 2 * g + jj
                                nc.tensor.matmul(
                                    t[:, jj * NQ:(jj + 1) * NQ],
                                    kT[h][:, j * P:(j + 1) * P],
                                    qs, start=True, stop=True)
                            return t

                        def do_exp(g, t):
                            if masked:
                                for jj in range(2):
                                    j = 2 * g + jj
                                    nc.vector.tensor_add(
                                        t[:, jj * NQ:(jj + 1) * NQ],
                                        t[:, jj * NQ:(jj + 1) * NQ],
                                        mch[j][:])
                            p = pTp.tile([P, 2 * NQ], bf16, name="pT")
                            nc.scalar.activation(p[:], t[:], Exp)
                            pT[g] = p

                        def avpair(g):
                            for jj in range(2):
                                j = 2 * g + jj
                                nc.tensor.matmul(
                                    ps_v[:],
                                    vch[j][:, h * DH:(h + 1) * DH],
                                    pT[g][:, jj * NQ:(jj + 1) * NQ],
                                    start=(j == 0), stop=(j == LKC - 1))

                        st = [spair(0)]
                        st.append(spair(1))
                        for g in range(8):
                            do_exp(g, st[g])
                            if g + 2 < 8:
                                st.append(spair(g + 2))
                            fill(h * 8 + g)
                            avpair(g)
                            # denominator accumulation on DVE (bf16 2x)
                            p = pT[g]
                            if g == 0:
                                acc = dnp.tile([P, NQ], bf16, name="dacc")
                                nc.vector.tensor_add(
                                    acc[:], p[:, 0:NQ], p[:, NQ:2 * NQ])
                            else:
                                nc.vector.tensor_add(
                                    acc[:], acc[:], p[:, 0:NQ])
                                nc.vector.tensor_add(
                                    acc[:], acc[:], p[:, NQ:2 * NQ])
                        dbc = dnp.tile([P, NQ], f32, name="dbc")
                        nc.gpsimd.partition_all_reduce(
                            dbc[:], acc[:], channels=P,
                            reduce_op=bass_isa.ReduceOp.add)
                        rr = dnp.tile([P, NQ], f32, name="rr")
                        nc.vector.reciprocal(rr[:], dbc[:])
                        nc.vector.tensor_mul(
                            valsT[h][:, q2 * NQ:(q2 + 1) * NQ],
                            ps_v[:], rr[:])

                ostate = {}

                def out_dst(q2, c):
                    lqc = q2 * 4 + c // 2
                    o2 = c % 2
                    return out_d[lqc * P:(lqc + 1) * P,
                                 o2 * NQ:(o2 + 1) * NQ]

                def op_part(q2, c, hh, start, stop):
                    """One head-part of output-projection chain (q2, c)."""
                    lqc = q2 * 4 + c // 2
                    o2 = c % 2
                    key = (q2, c)
                    if start:
                        ostate[key] = psx.tile([P, NQ], f32, name="px")
                    nc.tensor.matmul(
                        ostate[key][:],
                        valsT[hh][:, lqc * P:(lqc + 1) * P],
                        woch[hh][:, o2 * NQ:(o2 + 1) * NQ],
                        start=start, stop=stop)

                def op_store(q2, c, split=False):
                    po = ostate[(q2, c)]
                    dst = out_dst(q2, c)
                    if split:
                        for half in range(2):
                            sl = slice(half * (NQ // 2), (half + 1) * (NQ // 2))
                            ot = osb.tile([P, NQ // 2], f32, name="ot")
                            nc.vector.tensor_copy(ot[:], po[:, sl])
                            nc.sync.dma_start(dst[:, sl], ot[:])
                    else:
                        ot = osb.tile([P, NQ], f32, name="ot")
                        nc.vector.tensor_copy(ot[:], po[:])
                        nc.sync.dma_start(dst, ot[:])

                def b_filler(c, hh):
                    """op0 chain part; full 4-head chain + store."""
                    def f():
                        op_part(0, c, hh, start=(hh == 0), stop=(hh == 3))
                        if hh == 3:
                            op_store(0, c)
                    return f

                def cA_filler(c, hh):
                    """op1 passA: heads 0-1 accumulate, partial to SBUF."""
                    def f():
                        op_part(1, c, hh, start=(hh == 0), stop=(hh == 1))
                        if hh == 1:
                            nc.vector.tensor_copy(
                                partial[c][:], ostate[(1, c)][:])
                    return f

                def cB_part(c, hh):
                    """op1 passB: heads 2-3 accumulate in a fresh group."""
                    op_part(1, c, hh, start=(hh == 2), stop=(hh == 3))

                def cB_fin(c, split=False):
                    po = ostate[(1, c)]
                    dst = out_dst(1, c)
                    if split:
                        for half in range(2):
                            sl = slice(half * (NQ // 2), (half + 1) * (NQ // 2))
                            ot = osb.tile([P, NQ // 2], f32, name="ot")
                            nc.vector.tensor_add(
                                ot[:], po[:, sl], partial[c][:, sl])
                            nc.sync.dma_start(dst[:, sl], ot[:])
                    else:
                        ot = osb.tile([P, NQ], f32, name="ot")
                        nc.vector.tensor_add(ot[:], po[:], partial[c][:])
                        nc.sync.dma_start(dst, ot[:])

                def a_filler(hh, d):
                    """q-projection (half 1) d-chain part for head hh."""
                    def f():
                        key = ("q", hh)
                        if d == 0:
                            ostate[key] = psx.tile([P, NQ], f32, name="px")
                        nc.tensor.matmul(
                            ostate[key][:],
                            wqc[d][:, hh * DH:(hh + 1) * DH],
                            xg1[d],
                            start=(d == 0), stop=(d == KC - 1))
                        if d == KC - 1:
                            nc.vector.tensor_scalar_add(
                                qT[hh][:, NQ:2 * NQ], ostate[key][:],
                                bq_sb[:, hh:hh + 1])
                    return f

                # q-half 0: fillers are the 4 q-proj(g1) head chains.
                slots0 = [[a_filler(s // 8, s % 8)] for s in range(32)]
                attn_q2(0, slots0)

                # q-half 1: op0 chains B0-B5 in loops h0-h2; op1 passA as
                # second fillers in h2 and single fillers in h3; B6/B7 are
                # reserved for the post-loop norm wait.
                slots1 = [[] for _ in range(32)]
                for c in range(6):
                    base = c * 4
                    for hh in range(4):
                        slots1[base + hh].append(b_filler(c, hh))
                for c in range(4):
                    slots1[16 + 2 * c].append(cA_filler(c, 0))
                    slots1[16 + 2 * c + 1].append(cA_filler(c, 1))
                for c in range(4, 8):
                    slots1[24 + 2 * (c - 4)].append(cA_filler(c, 0))
                    slots1[24 + 2 * (c - 4) + 1].append(cA_filler(c, 1))
                attn_q2(1, slots1)

                # Post-loop: reserved op0 chains cover the last norm wait,
                # then op1 passB drains with at most two chains in flight.
                for c in (6, 7):
                    for hh in range(4):
                        op_part(0, c, hh, start=(hh == 0), stop=(hh == 3))
                    op_store(0, c)
                cB_part(0, 2)
                cB_part(1, 2)
                for c in range(8):
                    cB_part(c, 3)
                    if c + 2 < 8:
                        cB_part(c + 2, 2)
                    cB_fin(c, split=(c == 7))

    nc.compile()
    return nc


def _get_built(masked):
    if masked not in _BUILT:
        _BUILT[masked] = _build(masked)
    return _BUILT[masked]


def _chunk_major(a, n_chunks):
    """[n_chunks*P, C] -> [P, n_chunks*C] with [p, d*C+c] = a[d*P+p, c]."""
    C = a.shape[1]
    return np.ascontiguousarray(
        a.reshape(n_chunks, P, C).transpose(1, 0, 2).reshape(P, n_chunks * C))


def _shard_inputs(inputs, masked):
    import ml_dtypes

    bf16 = ml_dtypes.bfloat16

    x = np.asarray(inputs["mhca_input"], np.float32)
    enc = np.asarray(inputs["encoder_output"], np.float32)
    mask = np.asarray(inputs["cross_mask"], np.float32)
    W_kv = np.asarray(inputs["W_kv"], np.float32)
    b_kv = np.asarray(inputs["b_kv"], np.float32)
    W_q = np.asarray(inputs["W_q"], np.float32)
    b_q = np.asarray(inputs["b_q"], np.float32)
    W_o = np.asarray(inputs["W_o"], np.float32)

    scale = 1.0 / math.sqrt(DH)
    in_maps = []
    for c in range(N_CORES):
        b = c // 2
        g = c % 2
        heads = list(range(g * HPC, (g + 1) * HPC))
        sl = slice(g * OQ, (g + 1) * OQ)
        k_rows = np.concatenate(
            [W_kv[h * 2 * DH:h * 2 * DH + DH] for h in heads], 0)
        v_rows = np.concatenate(
            [W_kv[h * 2 * DH + DH:(h + 1) * 2 * DH] for h in heads], 0)
        xT = x[b].T                                   # [D, LQ]
        encT = enc[b].T                               # [D, LK]
        m = {
            "xr": _chunk_major(
                np.concatenate([xT[:, :NQ], xT[:, NQ:]], 0), 2 * KC
            ).astype(bf16),
            "e0r": _chunk_major(encT[:, :HLK], KC).astype(bf16),
            "e1r": _chunk_major(encT[:, HLK:], KC).astype(bf16),
            "wqr": _chunk_major((W_q[sl] * scale).T, KC).astype(bf16),
            "wkr": _chunk_major(k_rows.T, KC).astype(bf16),
            "wvr": _chunk_major(v_rows.T, KC).astype(bf16),
            "wor": _chunk_major(np.ascontiguousarray(W_o[:, sl].T), HPC),
            "bq": np.ascontiguousarray((b_q[sl] * scale).reshape(HPC, DH).T),
            "bk": np.ascontiguousarray(
                np.stack([b_kv[h * 2 * DH:h * 2 * DH + DH] for h in heads], 1)),
        }
        if masked:
            m["maskT"] = np.ascontiguousarray(mask[b].T)
        in_maps.append(m)
    return in_maps


def kernel(mhca_input, encoder_output, cross_mask, W_kv, b_kv, W_q, b_q, W_o,
           b_o):
    from concourse.bass_utils import run_bass_kernel_spmd

    inputs = {
        "mhca_input": mhca_input, "encoder_output": encoder_output,
        "cross_mask": cross_mask, "W_kv": W_kv, "b_kv": b_kv, "W_q": W_q,
        "b_q": b_q, "W_o": W_o,
    }
    b_kv = np.asarray(b_kv, np.float32)
    b_o = np.asarray(b_o, np.float32)
    W_o_np = np.asarray(W_o, np.float32)
    # v-bias folds into the output bias: out += W_o @ b_v + b_o
    b_v_vec = np.concatenate(
        [b_kv[h * 2 * DH + DH:(h + 1) * 2 * DH] for h in range(H)], 0)
    b_eff = b_o + W_o_np @ b_v_vec
    masked = bool(np.any(np.asarray(cross_mask)))
    nc = _get_built(masked)
    in_maps = _shard_inputs(inputs, masked)

    res = run_bass_kernel_spmd(nc, in_maps, core_ids=list(range(N_CORES)))
    outs = [res.results[c]["out"] for c in range(N_CORES)]
    full = np.stack([outs[2 * b] + outs[2 * b + 1] for b in range(B)], 0)
    return (full + b_eff[None, None, :]).astype(np.float32)


# revision 9
# speedup vs baseline: 1.0338x; 1.0338x over previous
"""Multi-head cross-attention on 8 Trainium2 NeuronCores.

Problem shapes (hardcoded): B=4, Ld=1024, Le=2048, d_model=1024, 8 heads x 128.
Sharding: core c handles batch b=c//2 and head-group g=c%2 (4 heads each).
Each core computes q/k/v projections for its heads, attention, and a partial
output projection over its heads' value dims; the host sums the two partial
outputs per batch and adds the (bias-folded) output bias.

Key scheduling/engine decisions:
- Projection inputs (enc, x, wq, wk, wv) ship as bf16 in chunk-major host
  layout ([128, chunks*cols]), so each tensor is one large contiguous DMA;
  only the cold-start K-proj inputs stream per chunk.
- Projections run d-major across all 8 PSUM banks; bank copy-backs alternate
  DVE/Act so the next phase's bank-0 chain never waits.
- Softmax denominators never touch the PE: exp'd score chunks (bf16)
  accumulate on DVE (2x mode), then one gpsimd partition_all_reduce per
  (q-half, head) produces the broadcast denominator, reciprocal on DVE.
- exp processes two key-chunks per Act instruction ([128,1024] spanning two
  PSUM banks) to keep Act throughput above the PE's score+AV rate.
- b_v is folded into the host-side output bias (b_eff = b_o + W_o @ b_v),
  removing all bias work from the attention loop.
- Attention is software-pipelined: one filler matmul per inner step keeps
  the PE busy while Act computes exp. Fillers for q-half 0 are the second
  half of the Q projection; fillers for q-half 1 are q-half 0's output
  projection. The PE is warmed with dummy matmuls on a memset tile so the
  p-state ramp completes before real work arrives.
"""

import math
import sys

import numpy as np

for _p in ("/opt/trn_rl_repo", "/root/.axon_site/_ro/trn_rl_repo"):
    if _p not in sys.path:
        sys.path.append(_p)

B = 4
LQ = 1024
LK = 2048
D = 1024
H = 8
DH = 128
P = 128
HPC = 4          # heads per core
OQ = HPC * DH    # 512 projected dims per core
NQ = 512         # matmul moving free dim
KC = D // P      # 8 contraction chunks for projections
LKC = LK // P    # 16 key chunks
HLK = LK // 2    # 1024, one lk-half of the encoder
N_CORES = 8

_BUILT = {}


def _build(masked):
    import concourse.bass as bass  # noqa: F401
    import concourse.tile as tile
    import concourse.mybir as mybir
    from concourse import bacc
    from concourse import bass_isa

    f32 = mybir.dt.float32
    f32r = mybir.dt.float32r
    bf16 = mybir.dt.bfloat16
    Exp = mybir.ActivationFunctionType.Exp

    nc = bacc.Bacc("TRN2", target_bir_lowering=False, debug=False,
                   num_devices=N_CORES)

    # chunk-major DRAM layouts: [...][p, d*cols + c] = chunk d, row p, col c
    xr = nc.dram_tensor("xr", [P, 2 * KC * NQ], bf16, kind="ExternalInput").ap()
    e0r = nc.dram_tensor("e0r", [P, KC * HLK], bf16, kind="ExternalInput").ap()
    e1r = nc.dram_tensor("e1r", [P, KC * HLK], bf16, kind="ExternalInput").ap()
    wqr = nc.dram_tensor("wqr", [P, KC * OQ], bf16, kind="ExternalInput").ap()
    wkr = nc.dram_tensor("wkr", [P, KC * OQ], bf16, kind="ExternalInput").ap()
    wvr = nc.dram_tensor("wvr", [P, KC * OQ], bf16, kind="ExternalInput").ap()
    wor = nc.dram_tensor("wor", [P, HPC * D], f32r, kind="ExternalInput").ap()
    bq_d = nc.dram_tensor("bq", [P, HPC], f32, kind="ExternalInput").ap()
    bk_d = nc.dram_tensor("bk", [P, HPC], f32, kind="ExternalInput").ap()
    if masked:
        maskT = nc.dram_tensor("maskT", [LK, LQ], f32, kind="ExternalInput").ap()
    out_d = nc.dram_tensor("out", [LQ, D], f32, kind="ExternalOutput").ap()

    with tile.TileContext(nc) as tc:
        with tc.tile_pool(name="persist", bufs=1) as persist:
            kT = [persist.tile([P, LK], f32r, name=f"kT{h}") for h in range(HPC)]
            qT = [persist.tile([P, LQ], f32r, name=f"qT{h}") for h in range(HPC)]
            vch = [persist.tile([P, OQ], bf16, name=f"v{j}") for j in range(LKC)]
            wq_all = persist.tile([P, KC * OQ], bf16, name="wq")
            wo_all = persist.tile([P, HPC * D], f32r, name="wo")
            x_all = persist.tile([P, 2 * KC * NQ], bf16, name="x")
            bq_sb = persist.tile([P, HPC], f32, name="bq")
            bk_sb = persist.tile([P, HPC], f32, name="bk")
            warm = persist.tile([P, 256], bf16, name="warm")

            wqc = [wq_all[:, d * OQ:(d + 1) * OQ] for d in range(KC)]
            woch = [wo_all[:, h * D:(h + 1) * D] for h in range(HPC)]
            xg0 = [x_all[:, d * NQ:(d + 1) * NQ] for d in range(KC)]
            xg1 = [x_all[:, (KC + d) * NQ:(KC + d + 1) * NQ] for d in range(KC)]

            with (
                tc.tile_pool(name="acc", bufs=1, space="PSUM") as accp,
                tc.tile_pool(name="wk", bufs=1) as wkp,
                tc.tile_pool(name="wv", bufs=1) as wvp,
                tc.tile_pool(name="e0p", bufs=1) as e0p,
                tc.tile_pool(name="e1p", bufs=1) as e1p,
            ):
                banks = [accp.tile([P, NQ], f32, name=f"bank{t}")
                         for t in range(8)]
                wk_all = wkp.tile([P, KC * OQ], bf16, name="wk")
                wv_all = wvp.tile([P, KC * OQ], bf16, name="wv")
                e0_all = e0p.tile([P, KC * HLK], bf16, name="e0")
                e1_all = e1p.tile([P, KC * HLK], bf16, name="e1")
                wkc = [wk_all[:, d * OQ:(d + 1) * OQ] for d in range(KC)]
                wvc = [wv_all[:, d * OQ:(d + 1) * OQ] for d in range(KC)]
                e0 = [e0_all[:, d * HLK:(d + 1) * HLK] for d in range(KC)]
                e1 = [e1_all[:, d * HLK:(d + 1) * HLK] for d in range(KC)]

                # --- PE warm-up: memset a tile (no DMA) and run dummy
                # matmuls so the p-state ramp happens before real work.
                nc.vector.memset(warm[:], 1.0)
                for _ in range(14):
                    nc.tensor.matmul(banks[0][0:16, 0:256], warm[:, 0:16],
                                     warm[:], start=True, stop=True)

                # --- DMA program: cold-start K inputs per chunk, the rest
                # as single contiguous transfers in need-order.
                for d in range(KC):
                    nc.sync.dma_start(wkc[d], wkr[:, d * OQ:(d + 1) * OQ])
                    nc.sync.dma_start(e0[d], e0r[:, d * HLK:(d + 1) * HLK])
                nc.sync.dma_start(bq_sb[:], bq_d[:])
                nc.sync.dma_start(bk_sb[:], bk_d[:])
                nc.sync.dma_start(wv_all[:], wvr[:])
                nc.sync.dma_start(e1_all[:], e1r[:])
                nc.sync.dma_start(wq_all[:], wqr[:])
                nc.sync.dma_start(x_all[:], xr[:])
                nc.sync.dma_start(wo_all[:], wor[:])

                def kproj_half(e, lh):
                    for d in range(KC):
                        for g in range(2):
                            for h in range(HPC):
                                nc.tensor.matmul(
                                    banks[g * 4 + h][:],
                                    wkc[d][:, h * DH:(h + 1) * DH],
                                    e[d][:, g * NQ:(g + 1) * NQ],
                                    start=(d == 0), stop=(d == KC - 1))
                    for i in range(8):
                        g, h = i // 4, i % 4
                        dst = kT[h][:, lh * HLK + g * NQ:lh * HLK + (g + 1) * NQ]
                        if i % 2 == 0:
                            nc.vector.tensor_scalar_add(
                                dst, banks[i][:], bk_sb[:, h:h + 1])
                        else:
                            nc.scalar.add(dst, banks[i][:], bk_sb[:, h:h + 1])

                def vproj_half(e, lh):
                    for d in range(KC):
                        for j8 in range(8):
                            nc.tensor.matmul(
                                banks[j8][:],
                                e[d][:, j8 * P:(j8 + 1) * P],
                                wvc[d],
                                start=(d == 0), stop=(d == KC - 1))
                    for j8 in range(8):
                        dst = vch[lh * 8 + j8][:]
                        if j8 % 2 == 0:
                            nc.vector.tensor_copy(dst, banks[j8][:])
                        else:
                            nc.scalar.copy(dst, banks[j8][:])

                kproj_half(e0, 0)
                vproj_half(e0, 0)
                kproj_half(e1, 1)
                vproj_half(e1, 1)

                # Q proj, query half 0, in two 2-head subphases so the
                # head-0/1 copy-backs finish before attention needs qT.
                for sub in range(2):
                    hs = (0, 1) if sub == 0 else (2, 3)
                    for d in range(KC):
                        for h in hs:
                            nc.tensor.matmul(
                                banks[h][:],
                                wqc[d][:, h * DH:(h + 1) * DH],
                                xg0[d],
                                start=(d == 0), stop=(d == KC - 1))
                    for h in hs:
                        nc.scalar.add(qT[h][:, 0:NQ], banks[h][:],
                                      bq_sb[:, h:h + 1])

            # ---- Attention, software-pipelined with filler matmuls.
            with (
                tc.tile_pool(name="pTp", bufs=12) as pTp,
                tc.tile_pool(name="dnp", bufs=2) as dnp,
                tc.tile_pool(name="maskp", bufs=16 if masked else 1) as maskp,
                tc.tile_pool(name="osb", bufs=4) as osb,
                tc.tile_pool(name="att", bufs=1) as attp,
                tc.tile_pool(name="pss", bufs=2, space="PSUM") as pss,
                tc.tile_pool(name="psa", bufs=2, space="PSUM") as psa,
                tc.tile_pool(name="psx", bufs=2, space="PSUM") as psx,
            ):
                valsT = [attp.tile([P, LQ], f32r, name=f"valsT{h}")
                         for h in range(HPC)]
                partial = [attp.tile([P, NQ], f32, name=f"prt{c}")
                           for c in range(8)]

                def attn_q2(q2, slot_fillers):
                    """h-loops for one query half. slot_fillers: 32 lists of
                    callables (slot = h*8+g), each emitting one PE matmul
                    (+ its own non-PE follow-ups)."""

                    def fill(slot):
                        for f in slot_fillers[slot]:
                            f()

                    if masked:
                        mch = []
                        for j in range(LKC):
                            mt = maskp.tile([P, NQ], f32, name=f"m{j}")
                            nc.sync.dma_start(
                                mt[:], maskT[j * P:(j + 1) * P,
                                             q2 * NQ:(q2 + 1) * NQ])
                            mch.append(mt)

                    for h in range(HPC):
                        qs = qT[h][:, q2 * NQ:(q2 + 1) * NQ]
                        ps_v = psa.tile([P, NQ], f32, name="ps_v")
                        pT = [None] * 8
                        acc = None

                        def spair(g):
                            t = pss.tile([P, 2 * NQ], f32, name="ps_s")
                            for jj in range(2):
                                j = 2 * g + jj
                                nc.tensor.matmul(
                                    t[:, jj * NQ:(jj + 1) * NQ],
                                    kT[h][:, j * P:(j + 1) * P],
                                    qs, start=True, stop=True)
                            return t

                        def do_exp(g, t):
                            if masked:
                                for jj in range(2):
                                    j = 2 * g + jj
                                    nc.vector.tensor_add(
                                        t[:, jj * NQ:(jj + 1) * NQ],
                                        t[:, jj * NQ:(jj + 1) * NQ],
                                        mch[j][:])
                            p = pTp.tile([P, 2 * NQ], bf16, name="pT")
                            nc.scalar.activation(p[:], t[:], Exp)
                            pT[g] = p

                        def avpair(g):
                            for jj in range(2):
                                j = 2 * g + jj
                                nc.tensor.matmul(
                                    ps_v[:],
                                    vch[j][:, h * DH:(h + 1) * DH],
                                    pT[g][:, jj * NQ:(jj + 1) * NQ],
                                    start=(j == 0), stop=(j == LKC - 1))

                        accp_ = None
                        st = [spair(0)]
                        st.append(spair(1))
                        for g in range(8):
                            do_exp(g, st[g])
                            if g + 2 < 8:
                                st.append(spair(g + 2))
                            fill(h * 8 + g)
                            avpair(g)
                            # denominator accumulation: DVE takes pairs
                            # 0,4..7 (bf16 2x), the idle Pool engine takes
                            # pairs 1..3; merged before the all-reduce.
                            p = pT[g]
                            if g == 0:
                                acc = dnp.tile([P, NQ], bf16, name="dacc")
                                nc.vector.tensor_add(
                                    acc[:], p[:, 0:NQ], p[:, NQ:2 * NQ])
                            elif g == 1:
                                accp_ = dnp.tile([P, NQ], bf16, name="daccp")
                                nc.gpsimd.tensor_add(
                                    accp_[:], p[:, 0:NQ], p[:, NQ:2 * NQ])
                            elif g <= 3:
                                nc.gpsimd.tensor_add(
                                    accp_[:], accp_[:], p[:, 0:NQ])
                                nc.gpsimd.tensor_add(
                                    accp_[:], accp_[:], p[:, NQ:2 * NQ])
                            else:
                                nc.vector.tensor_add(
                                    acc[:], acc[:], p[:, 0:NQ])
                                nc.vector.tensor_add(
                                    acc[:], acc[:], p[:, NQ:2 * NQ])
                        nc.vector.tensor_add(acc[:], acc[:], accp_[:])
                        dbc = dnp.tile([P, NQ], f32, name="dbc")
                        nc.gpsimd.partition_all_reduce(
                            dbc[:], acc[:], channels=P,
                            reduce_op=bass_isa.ReduceOp.add)
                        rr = dnp.tile([P, NQ], f32, name="rr")
                        nc.vector.reciprocal(rr[:], dbc[:])
                        nc.vector.tensor_mul(
                            valsT[h][:, q2 * NQ:(q2 + 1) * NQ],
                            ps_v[:], rr[:])

                ostate = {}

                def out_dst(q2, c):
                    lqc = q2 * 4 + c // 2
                    o2 = c % 2
                    return out_d[lqc * P:(lqc + 1) * P,
                                 o2 * NQ:(o2 + 1) * NQ]

                def op_part(q2, c, hh, start, stop):
                    """One head-part of output-projection chain (q2, c)."""
                    lqc = q2 * 4 + c // 2
                    o2 = c % 2
                    key = (q2, c)
                    if start:
                        ostate[key] = psx.tile([P, NQ], f32, name="px")
                    nc.tensor.matmul(
                        ostate[key][:],
                        valsT[hh][:, lqc * P:(lqc + 1) * P],
                        woch[hh][:, o2 * NQ:(o2 + 1) * NQ],
                        start=start, stop=stop)

                def op_store(q2, c, split=False, pool_copy=False):
                    po = ostate[(q2, c)]
                    dst = out_dst(q2, c)
                    if split:
                        for half in range(2):
                            sl = slice(half * (NQ // 2), (half + 1) * (NQ // 2))
                            ot = osb.tile([P, NQ // 2], f32, name="ot")
                            nc.vector.tensor_copy(ot[:], po[:, sl])
                            nc.sync.dma_start(dst[:, sl], ot[:])
                    else:
                        ot = osb.tile([P, NQ], f32, name="ot")
                        nc.vector.tensor_copy(ot[:], po[:])
                        nc.sync.dma_start(dst, ot[:])

                def b_filler(c, hh):
                    """op0 chain part; full 4-head chain + store."""
                    def f():
                        op_part(0, c, hh, start=(hh == 0), stop=(hh == 3))
                        if hh == 3:
                            op_store(0, c)
                    return f

                def cA_filler(c, hh):
                    """op1 passA: heads 0-1 accumulate, partial to SBUF."""
                    def f():
                        op_part(1, c, hh, start=(hh == 0), stop=(hh == 1))
                        if hh == 1:
                            nc.vector.tensor_copy(
                                partial[c][:], ostate[(1, c)][:])
                    return f

                def cB_part(c, hh):
                    """op1 passB: heads 2-3 accumulate in a fresh group."""
                    op_part(1, c, hh, start=(hh == 2), stop=(hh == 3))

                def cB_fin(c, split=False):
                    po = ostate[(1, c)]
                    dst = out_dst(1, c)
                    if split:
                        for half in range(2):
                            sl = slice(half * (NQ // 2), (half + 1) * (NQ // 2))
                            ot = osb.tile([P, NQ // 2], f32, name="ot")
                            nc.vector.tensor_add(
                                ot[:], po[:, sl], partial[c][:, sl])
                            nc.sync.dma_start(dst[:, sl], ot[:])
                    else:
                        ot = osb.tile([P, NQ], f32, name="ot")
                        nc.vector.tensor_add(ot[:], po[:], partial[c][:])
                        nc.sync.dma_start(dst, ot[:])

                def a_filler(hh, d):
                    """q-projection (half 1) d-chain part for head hh."""
                    def f():
                        key = ("q", hh)
                        if d == 0:
                            ostate[key] = psx.tile([P, NQ], f32, name="px")
                        nc.tensor.matmul(
                            ostate[key][:],
                            wqc[d][:, hh * DH:(hh + 1) * DH],
                            xg1[d],
                            start=(d == 0), stop=(d == KC - 1))
                        if d == KC - 1:
                            nc.vector.tensor_scalar_add(
                                qT[hh][:, NQ:2 * NQ], ostate[key][:],
                                bq_sb[:, hh:hh + 1])
                    return f

                # q-half 0: fillers are the 4 q-proj(g1) head chains.
                slots0 = [[a_filler(s // 8, s % 8)] for s in range(32)]
                attn_q2(0, slots0)

                # q-half 1: op0 chains B0-B5 in loops h0-h2; op1 passA as
                # second fillers in h2 and single fillers in h3; B6/B7 are
                # reserved for the post-loop norm wait.
                slots1 = [[] for _ in range(32)]
                for c in range(6):
                    base = c * 4
                    for hh in range(4):
                        slots1[base + hh].append(b_filler(c, hh))
                for c in range(4):
                    slots1[16 + 2 * c].append(cA_filler(c, 0))
                    slots1[16 + 2 * c + 1].append(cA_filler(c, 1))
                for c in range(4, 8):
                    slots1[24 + 2 * (c - 4)].append(cA_filler(c, 0))
                    slots1[24 + 2 * (c - 4) + 1].append(cA_filler(c, 1))
                attn_q2(1, slots1)

                # Post-loop: reserved op0 chains cover the last norm wait,
                # then op1 passB drains with at most two chains in flight.
                for c in (6, 7):
                    for hh in range(4):
                        op_part(0, c, hh, start=(hh == 0), stop=(hh == 3))
                    op_store(0, c)
                cB_part(0, 2)
                cB_part(1, 2)
                for c in range(8):
                    cB_part(c, 3)
                    if c + 2 < 8:
                        cB_part(c + 2, 2)
                    cB_fin(c, split=(c == 7))

    nc.compile()
    return nc


def _get_built(masked):
    if masked not in _BUILT:
        _BUILT[masked] = _build(masked)
    return _BUILT[masked]


def _chunk_major(a, n_chunks):
    """[n_chunks*P, C] -> [P, n_chunks*C] with [p, d*C+c] = a[d*P+p, c]."""
    C = a.shape[1]
    return np.ascontiguousarray(
        a.reshape(n_chunks, P, C).transpose(1, 0, 2).reshape(P, n_chunks * C))


def _shard_inputs(inputs, masked):
    import ml_dtypes

    bf16 = ml_dtypes.bfloat16

    x = np.asarray(inputs["mhca_input"], np.float32)
    enc = np.asarray(inputs["encoder_output"], np.float32)
    mask = np.asarray(inputs["cross_mask"], np.float32)
    W_kv = np.asarray(inputs["W_kv"], np.float32)
    b_kv = np.asarray(inputs["b_kv"], np.float32)
    W_q = np.asarray(inputs["W_q"], np.float32)
    b_q = np.asarray(inputs["b_q"], np.float32)
    W_o = np.asarray(inputs["W_o"], np.float32)

    scale = 1.0 / math.sqrt(DH)
    in_maps = []
    for c in range(N_CORES):
        b = c // 2
        g = c % 2
        heads = list(range(g * HPC, (g + 1) * HPC))
        sl = slice(g * OQ, (g + 1) * OQ)
        k_rows = np.concatenate(
            [W_kv[h * 2 * DH:h * 2 * DH + DH] for h in heads], 0)
        v_rows = np.concatenate(
            [W_kv[h * 2 * DH + DH:(h + 1) * 2 * DH] for h in heads], 0)
        xT = x[b].T                                   # [D, LQ]
        encT = enc[b].T                               # [D, LK]
        m = {
            "xr": _chunk_major(
                np.concatenate([xT[:, :NQ], xT[:, NQ:]], 0), 2 * KC
            ).astype(bf16),
            "e0r": _chunk_major(encT[:, :HLK], KC).astype(bf16),
            "e1r": _chunk_major(encT[:, HLK:], KC).astype(bf16),
            "wqr": _chunk_major((W_q[sl] * scale).T, KC).astype(bf16),
            "wkr": _chunk_major(k_rows.T, KC).astype(bf16),
            "wvr": _chunk_major(v_rows.T, KC).astype(bf16),
            "wor": _chunk_major(np.ascontiguousarray(W_o[:, sl].T), HPC),
            "bq": np.ascontiguousarray((b_q[sl] * scale).reshape(HPC, DH).T),
            "bk": np.ascontiguousarray(
                np.stack([b_kv[h * 2 * DH:h * 2 * DH + DH] for h in heads], 1)),
        }
        if masked:
            m["maskT"] = np.ascontiguousarray(mask[b].T)
        in_maps.append(m)
    return in_maps


def kernel(mhca_input, encoder_output, cross_mask, W_kv, b_kv, W_q, b_q, W_o,
           b_o):
    from concourse.bass_utils import run_bass_kernel_spmd

    inputs = {
        "mhca_input": mhca_input, "encoder_output": encoder_output,
        "cross_mask": cross_mask, "W_kv": W_kv, "b_kv": b_kv, "W_q": W_q,
        "b_q": b_q, "W_o": W_o,
    }
    b_kv = np.asarray(b_kv, np.float32)
    b_o = np.asarray(b_o, np.float32)
    W_o_np = np.asarray(W_o, np.float32)
    # v-bias folds into the output bias: out += W_o @ b_v + b_o
    b_v_vec = np.concatenate(
        [b_kv[h * 2 * DH + DH:(h + 1) * 2 * DH] for h in range(H)], 0)
    b_eff = b_o + W_o_np @ b_v_vec
    masked = bool(np.any(np.asarray(cross_mask)))
    nc = _get_built(masked)
    in_maps = _shard_inputs(inputs, masked)

    res = run_bass_kernel_spmd(nc, in_maps, core_ids=list(range(N_CORES)))
    outs = [res.results[c]["out"] for c in range(N_CORES)]
    full = np.stack([outs[2 * b] + outs[2 * b + 1] for b in range(B)], 0)
    return (full + b_eff[None, None, :]).astype(np.float32)


# revision 10
# speedup vs baseline: 1.0580x; 1.0235x over previous
"""Multi-head cross-attention on 8 Trainium2 NeuronCores.

Problem shapes (hardcoded): B=4, Ld=1024, Le=2048, d_model=1024, 8 heads x 128.
Sharding: core c handles batch b=c//2 and head-group g=c%2 (4 heads each).
Each core computes q/k/v projections for its heads, attention, and a partial
output projection over its heads' value dims; the host sums the two partial
outputs per batch and adds the (bias-folded) output bias.

Key scheduling/engine decisions:
- Projection inputs (enc, x, wq, wk, wv) ship as bf16 in chunk-major host
  layout ([128, chunks*cols]), so each tensor is one large contiguous DMA;
  only the cold-start K-proj inputs stream per chunk.
- Projections run d-major across all 8 PSUM banks; bank copy-backs alternate
  DVE/Act so the next phase's bank-0 chain never waits.
- Softmax denominators never touch the PE: exp'd score chunks (bf16)
  accumulate on DVE (2x mode), then one gpsimd partition_all_reduce per
  (q-half, head) produces the broadcast denominator, reciprocal on DVE.
- exp processes two key-chunks per Act instruction ([128,1024] spanning two
  PSUM banks) to keep Act throughput above the PE's score+AV rate.
- b_v is folded into the host-side output bias (b_eff = b_o + W_o @ b_v),
  removing all bias work from the attention loop.
- Attention is software-pipelined: one filler matmul per inner step keeps
  the PE busy while Act computes exp. Fillers for q-half 0 are the second
  half of the Q projection; fillers for q-half 1 are q-half 0's output
  projection. The PE is warmed with dummy matmuls on a memset tile so the
  p-state ramp completes before real work arrives.
"""

import math
import sys

import numpy as np

for _p in ("/opt/trn_rl_repo", "/root/.axon_site/_ro/trn_rl_repo"):
    if _p not in sys.path:
        sys.path.append(_p)

B = 4
LQ = 1024
LK = 2048
D = 1024
H = 8
DH = 128
P = 128
HPC = 4          # heads per core
OQ = HPC * DH    # 512 projected dims per core
NQ = 512         # matmul moving free dim
KC = D // P      # 8 contraction chunks for projections
LKC = LK // P    # 16 key chunks
HLK = LK // 2    # 1024, one lk-half of the encoder
N_CORES = 8

_BUILT = {}


def _build(masked):
    import concourse.bass as bass  # noqa: F401
    import concourse.tile as tile
    import concourse.mybir as mybir
    from concourse import bacc
    from concourse import bass_isa

    f32 = mybir.dt.float32
    f32r = mybir.dt.float32r
    bf16 = mybir.dt.bfloat16
    Exp = mybir.ActivationFunctionType.Exp

    nc = bacc.Bacc("TRN2", target_bir_lowering=False, debug=False,
                   num_devices=N_CORES)

    # chunk-major DRAM layouts: [...][p, d*cols + c] = chunk d, row p, col c
    xr = nc.dram_tensor("xr", [P, 2 * KC * NQ], bf16, kind="ExternalInput").ap()
    e0r = nc.dram_tensor("e0r", [P, KC * HLK], bf16, kind="ExternalInput").ap()
    e1r = nc.dram_tensor("e1r", [P, KC * HLK], bf16, kind="ExternalInput").ap()
    wqr = nc.dram_tensor("wqr", [P, KC * OQ], bf16, kind="ExternalInput").ap()
    wkr = nc.dram_tensor("wkr", [P, KC * OQ], bf16, kind="ExternalInput").ap()
    wvr = nc.dram_tensor("wvr", [P, KC * OQ], bf16, kind="ExternalInput").ap()
    wor = nc.dram_tensor("wor", [P, HPC * D], f32r, kind="ExternalInput").ap()
    bq_d = nc.dram_tensor("bq", [P, HPC], f32, kind="ExternalInput").ap()
    bk_d = nc.dram_tensor("bk", [P, HPC], f32, kind="ExternalInput").ap()
    if masked:
        maskT = nc.dram_tensor("maskT", [LK, LQ], f32, kind="ExternalInput").ap()
    out_d = nc.dram_tensor("out", [LQ, D], f32, kind="ExternalOutput").ap()

    with tile.TileContext(nc) as tc:
        with tc.tile_pool(name="persist", bufs=1) as persist:
            kT = [persist.tile([P, LK], f32r, name=f"kT{h}") for h in range(HPC)]
            qT = [persist.tile([P, LQ], f32r, name=f"qT{h}") for h in range(HPC)]
            vch = [persist.tile([P, OQ], bf16, name=f"v{j}") for j in range(LKC)]
            wq_all = persist.tile([P, KC * OQ], bf16, name="wq")
            wo_all = persist.tile([P, HPC * D], f32r, name="wo")
            x_all = persist.tile([P, 2 * KC * NQ], bf16, name="x")
            bq_sb = persist.tile([P, HPC], f32, name="bq")
            bk_sb = persist.tile([P, HPC], f32, name="bk")
            warm = persist.tile([P, 256], bf16, name="warm")

            wqc = [wq_all[:, d * OQ:(d + 1) * OQ] for d in range(KC)]
            woch = [wo_all[:, h * D:(h + 1) * D] for h in range(HPC)]
            xg0 = [x_all[:, d * NQ:(d + 1) * NQ] for d in range(KC)]
            xg1 = [x_all[:, (KC + d) * NQ:(KC + d + 1) * NQ] for d in range(KC)]

            with (
                tc.tile_pool(name="acc", bufs=1, space="PSUM") as accp,
                tc.tile_pool(name="wk", bufs=1) as wkp,
                tc.tile_pool(name="wv", bufs=1) as wvp,
                tc.tile_pool(name="e0p", bufs=1) as e0p,
                tc.tile_pool(name="e1p", bufs=1) as e1p,
            ):
                banks = [accp.tile([P, NQ], f32, name=f"bank{t}")
                         for t in range(8)]
                wk_all = wkp.tile([P, KC * OQ], bf16, name="wk")
                wv_all = wvp.tile([P, KC * OQ], bf16, name="wv")
                e0_all = e0p.tile([P, KC * HLK], bf16, name="e0")
                e1_all = e1p.tile([P, KC * HLK], bf16, name="e1")
                wkc = [wk_all[:, d * OQ:(d + 1) * OQ] for d in range(KC)]
                wvc = [wv_all[:, d * OQ:(d + 1) * OQ] for d in range(KC)]
                e0 = [e0_all[:, d * HLK:(d + 1) * HLK] for d in range(KC)]
                e1 = [e1_all[:, d * HLK:(d + 1) * HLK] for d in range(KC)]

                # --- PE warm-up: memset a tile (no DMA) and run dummy
                # matmuls so the p-state ramp happens before real work.
                nc.vector.memset(warm[:], 1.0)
                for _ in range(14):
                    nc.tensor.matmul(banks[0][0:16, 0:256], warm[:, 0:16],
                                     warm[:], start=True, stop=True)

                # --- DMA program: cold-start K inputs per chunk, the rest
                # as single contiguous transfers in need-order.
                for d in range(KC):
                    nc.sync.dma_start(wkc[d], wkr[:, d * OQ:(d + 1) * OQ])
                    nc.sync.dma_start(e0[d], e0r[:, d * HLK:(d + 1) * HLK])
                nc.sync.dma_start(bq_sb[:], bq_d[:])
                nc.sync.dma_start(bk_sb[:], bk_d[:])
                nc.sync.dma_start(wv_all[:], wvr[:])
                nc.sync.dma_start(e1_all[:], e1r[:])
                nc.sync.dma_start(wq_all[:], wqr[:])
                nc.sync.dma_start(x_all[:], xr[:])
                nc.sync.dma_start(wo_all[:], wor[:])

                def kproj_half(e, lh):
                    for d in range(KC):
                        for g in range(2):
                            for h in range(HPC):
                                nc.tensor.matmul(
                                    banks[g * 4 + h][:],
                                    wkc[d][:, h * DH:(h + 1) * DH],
                                    e[d][:, g * NQ:(g + 1) * NQ],
                                    start=(d == 0), stop=(d == KC - 1))
                    for i in range(8):
                        g, h = i // 4, i % 4
                        dst = kT[h][:, lh * HLK + g * NQ:lh * HLK + (g + 1) * NQ]
                        if i % 2 == 0:
                            nc.vector.tensor_scalar_add(
                                dst, banks[i][:], bk_sb[:, h:h + 1])
                        else:
                            nc.scalar.add(dst, banks[i][:], bk_sb[:, h:h + 1])

                def vproj_half(e, lh):
                    for d in range(KC):
                        for j8 in range(8):
                            nc.tensor.matmul(
                                banks[j8][:],
                                e[d][:, j8 * P:(j8 + 1) * P],
                                wvc[d],
                                start=(d == 0), stop=(d == KC - 1))
                    for j8 in range(8):
                        dst = vch[lh * 8 + j8][:]
                        if j8 % 2 == 0:
                            nc.vector.tensor_copy(dst, banks[j8][:])
                        else:
                            nc.scalar.copy(dst, banks[j8][:])

                kproj_half(e0, 0)
                vproj_half(e0, 0)
                kproj_half(e1, 1)
                vproj_half(e1, 1)

                # Q proj, query half 0, in two 2-head subphases so the
                # head-0/1 copy-backs finish before attention needs qT.
                for sub in range(2):
                    hs = (0, 1) if sub == 0 else (2, 3)
                    for d in range(KC):
                        for h in hs:
                            nc.tensor.matmul(
                                banks[h][:],
                                wqc[d][:, h * DH:(h + 1) * DH],
                                xg0[d],
                                start=(d == 0), stop=(d == KC - 1))
                    for h in hs:
                        nc.scalar.add(qT[h][:, 0:NQ], banks[h][:],
                                      bq_sb[:, h:h + 1])

            # ---- Attention, software-pipelined with filler matmuls.
            with (
                tc.tile_pool(name="pTp", bufs=12) as pTp,
                tc.tile_pool(name="dnp", bufs=2) as dnp,
                tc.tile_pool(name="maskp", bufs=16 if masked else 1) as maskp,
                tc.tile_pool(name="osb", bufs=4) as osb,
                tc.tile_pool(name="att", bufs=1) as attp,
                tc.tile_pool(name="pss", bufs=2, space="PSUM") as pss,
                tc.tile_pool(name="psa", bufs=2, space="PSUM") as psa,
                tc.tile_pool(name="psx", bufs=2, space="PSUM") as psx,
            ):
                valsT = [attp.tile([P, LQ], f32r, name=f"valsT{h}")
                         for h in range(HPC)]
                partial = [attp.tile([P, NQ], f32, name=f"prt{c}")
                           for c in range(8)]

                def attn_q2(q2, slot_fillers):
                    """h-loops for one query half. slot_fillers: 32 lists of
                    callables (slot = h*8+g), each emitting one PE matmul
                    (+ its own non-PE follow-ups)."""

                    def fill(slot):
                        for f in slot_fillers[slot]:
                            f()

                    if masked:
                        mch = []
                        for j in range(LKC):
                            mt = maskp.tile([P, NQ], f32, name=f"m{j}")
                            nc.sync.dma_start(
                                mt[:], maskT[j * P:(j + 1) * P,
                                             q2 * NQ:(q2 + 1) * NQ])
                            mch.append(mt)

                    for h in range(HPC):
                        qs = qT[h][:, q2 * NQ:(q2 + 1) * NQ]
                        ps_v = psa.tile([P, NQ], f32, name="ps_v")
                        pT = [None] * 8
                        acc = None

                        def spair(g):
                            t = pss.tile([P, 2 * NQ], f32, name="ps_s")
                            for jj in range(2):
                                j = 2 * g + jj
                                nc.tensor.matmul(
                                    t[:, jj * NQ:(jj + 1) * NQ],
                                    kT[h][:, j * P:(j + 1) * P],
                                    qs, start=True, stop=True)
                            return t

                        def do_exp(g, t):
                            if masked:
                                for jj in range(2):
                                    j = 2 * g + jj
                                    nc.vector.tensor_add(
                                        t[:, jj * NQ:(jj + 1) * NQ],
                                        t[:, jj * NQ:(jj + 1) * NQ],
                                        mch[j][:])
                            p = pTp.tile([P, 2 * NQ], bf16, name="pT")
                            nc.scalar.activation(p[:], t[:], Exp)
                            pT[g] = p

                        def avpair(g):
                            for jj in range(2):
                                j = 2 * g + jj
                                nc.tensor.matmul(
                                    ps_v[:],
                                    vch[j][:, h * DH:(h + 1) * DH],
                                    pT[g][:, jj * NQ:(jj + 1) * NQ],
                                    start=(j == 0), stop=(j == LKC - 1))

                        accp_ = None
                        st = [spair(0)]
                        st.append(spair(1))
                        for g in range(8):
                            do_exp(g, st[g])
                            if g + 2 < 8:
                                st.append(spair(g + 2))
                            fill(h * 8 + g)
                            avpair(g)
                            # denominator accumulation: DVE takes pairs
                            # 0,4..7 (bf16 2x), the idle Pool engine takes
                            # pairs 1..3; merged before the all-reduce.
                            p = pT[g]
                            if g == 0:
                                acc = dnp.tile([P, NQ], bf16, name="dacc")
                                nc.vector.tensor_add(
                                    acc[:], p[:, 0:NQ], p[:, NQ:2 * NQ])
                            elif g == 1:
                                accp_ = dnp.tile([P, NQ], bf16, name="daccp")
                                nc.gpsimd.tensor_add(
                                    accp_[:], p[:, 0:NQ], p[:, NQ:2 * NQ])
                            elif g <= 3:
                                nc.gpsimd.tensor_add(
                                    accp_[:], accp_[:], p[:, 0:NQ])
                                nc.gpsimd.tensor_add(
                                    accp_[:], accp_[:], p[:, NQ:2 * NQ])
                            else:
                                nc.vector.tensor_add(
                                    acc[:], acc[:], p[:, 0:NQ])
                                nc.vector.tensor_add(
                                    acc[:], acc[:], p[:, NQ:2 * NQ])
                        nc.vector.tensor_add(acc[:], acc[:], accp_[:])
                        dbc = dnp.tile([P, NQ], f32, name="dbc")
                        nc.gpsimd.partition_all_reduce(
                            dbc[:], acc[:], channels=P,
                            reduce_op=bass_isa.ReduceOp.add)
                        rr = dnp.tile([P, NQ], f32, name="rr")
                        nc.vector.reciprocal(rr[:], dbc[:])
                        nc.vector.tensor_mul(
                            valsT[h][:, q2 * NQ:(q2 + 1) * NQ],
                            ps_v[:], rr[:])

                ostate = {}

                def out_dst(q2, c):
                    lqc = q2 * 4 + c // 2
                    o2 = c % 2
                    return out_d[lqc * P:(lqc + 1) * P,
                                 o2 * NQ:(o2 + 1) * NQ]

                def op_part(q2, c, hh, start, stop):
                    """One head-part of output-projection chain (q2, c)."""
                    lqc = q2 * 4 + c // 2
                    o2 = c % 2
                    key = (q2, c)
                    if start:
                        ostate[key] = psx.tile([P, NQ], f32, name="px")
                    nc.tensor.matmul(
                        ostate[key][:],
                        valsT[hh][:, lqc * P:(lqc + 1) * P],
                        woch[hh][:, o2 * NQ:(o2 + 1) * NQ],
                        start=start, stop=stop)

                def op_store(q2, c, split=False, on_act=False):
                    po = ostate[(q2, c)]
                    dst = out_dst(q2, c)
                    if split:
                        for half in range(2):
                            sl = slice(half * (NQ // 2), (half + 1) * (NQ // 2))
                            ot = osb.tile([P, NQ // 2], f32, name="ot")
                            nc.vector.tensor_copy(ot[:], po[:, sl])
                            nc.sync.dma_start(dst[:, sl], ot[:])
                    else:
                        ot = osb.tile([P, NQ], f32, name="ot")
                        if on_act:
                            nc.scalar.copy(ot[:], po[:])
                        else:
                            nc.vector.tensor_copy(ot[:], po[:])
                        nc.sync.dma_start(dst, ot[:])

                def b_filler(c, hh):
                    """op0 chain part; full 4-head chain + store."""
                    def f():
                        op_part(0, c, hh, start=(hh == 0), stop=(hh == 3))
                        if hh == 3:
                            op_store(0, c)
                    return f

                def a_filler(hh, d):
                    """q-projection (half 1) d-chain part for head hh."""
                    def f():
                        key = ("q", hh)
                        if d == 0:
                            ostate[key] = psx.tile([P, NQ], f32, name="px")
                        nc.tensor.matmul(
                            ostate[key][:],
                            wqc[d][:, hh * DH:(hh + 1) * DH],
                            xg1[d],
                            start=(d == 0), stop=(d == KC - 1))
                        if d == KC - 1:
                            nc.vector.tensor_scalar_add(
                                qT[hh][:, NQ:2 * NQ], ostate[key][:],
                                bq_sb[:, hh:hh + 1])
                    return f

                # q-half 0: fillers are the 4 q-proj(g1) head chains.
                slots0 = [[a_filler(s // 8, s % 8)] for s in range(32)]
                attn_q2(0, slots0)

                # q-half 1: op0 chains B0-B4 fill loops h0-h2; B5-B7 are
                # reserved to cover the final norm chain post-loop.
                slots1 = [[] for _ in range(32)]
                for c in range(5):
                    base = c * 4
                    for hh in range(4):
                        slots1[base + hh].append(b_filler(c, hh))
                attn_q2(1, slots1)

                # Post-loop: reserves bridge the last norm wait, then the
                # final out-projection drains with copies split across
                # Act (idle once exps end) and DVE.
                for c in (5, 6, 7):
                    for hh in range(4):
                        op_part(0, c, hh, start=(hh == 0), stop=(hh == 3))
                    op_store(0, c, on_act=True)
                for c in range(8):
                    for hh in range(4):
                        op_part(1, c, hh, start=(hh == 0), stop=(hh == 3))
                    if c == 7:
                        op_store(1, c, split=True)
                    else:
                        op_store(1, c, on_act=(c % 2 == 1))

    nc.compile()
    return nc


def _get_built(masked):
    if masked not in _BUILT:
        _BUILT[masked] = _build(masked)
    return _BUILT[masked]


def _chunk_major(a, n_chunks):
    """[n_chunks*P, C] -> [P, n_chunks*C] with [p, d*C+c] = a[d*P+p, c]."""
    C = a.shape[1]
    return np.ascontiguousarray(
        a.reshape(n_chunks, P, C).transpose(1, 0, 2).reshape(P, n_chunks * C))


def _shard_inputs(inputs, masked):
    import ml_dtypes

    bf16 = ml_dtypes.bfloat16

    x = np.asarray(inputs["mhca_input"], np.float32)
    enc = np.asarray(inputs["encoder_output"], np.float32)
    mask = np.asarray(inputs["cross_mask"], np.float32)
    W_kv = np.asarray(inputs["W_kv"], np.float32)
    b_kv = np.asarray(inputs["b_kv"], np.float32)
    W_q = np.asarray(inputs["W_q"], np.float32)
    b_q = np.asarray(inputs["b_q"], np.float32)
    W_o = np.asarray(inputs["W_o"], np.float32)

    scale = 1.0 / math.sqrt(DH)
    in_maps = []
    for c in range(N_CORES):
        b = c // 2
        g = c % 2
        heads = list(range(g * HPC, (g + 1) * HPC))
        sl = slice(g * OQ, (g + 1) * OQ)
        k_rows = np.concatenate(
            [W_kv[h * 2 * DH:h * 2 * DH + DH] for h in heads], 0)
        v_rows = np.concatenate(
            [W_kv[h * 2 * DH + DH:(h + 1) * 2 * DH] for h in heads], 0)
        xT = x[b].T                                   # [D, LQ]
        encT = enc[b].T                               # [D, LK]
        m = {
            "xr": _chunk_major(
                np.concatenate([xT[:, :NQ], xT[:, NQ:]], 0), 2 * KC
            ).astype(bf16),
            "e0r": _chunk_major(encT[:, :HLK], KC).astype(bf16),
            "e1r": _chunk_major(encT[:, HLK:], KC).astype(bf16),
            "wqr": _chunk_major((W_q[sl] * scale).T, KC).astype(bf16),
            "wkr": _chunk_major(k_rows.T, KC).astype(bf16),
            "wvr": _chunk_major(v_rows.T, KC).astype(bf16),
            "wor": _chunk_major(np.ascontiguousarray(W_o[:, sl].T), HPC),
            "bq": np.ascontiguousarray((b_q[sl] * scale).reshape(HPC, DH).T),
            "bk": np.ascontiguousarray(
                np.stack([b_kv[h * 2 * DH:h * 2 * DH + DH] for h in heads], 1)),
        }
        if masked:
            m["maskT"] = np.ascontiguousarray(mask[b].T)
        in_maps.append(m)
    return in_maps


def kernel(mhca_input, encoder_output, cross_mask, W_kv, b_kv, W_q, b_q, W_o,
           b_o):
    from concourse.bass_utils import run_bass_kernel_spmd

    inputs = {
        "mhca_input": mhca_input, "encoder_output": encoder_output,
        "cross_mask": cross_mask, "W_kv": W_kv, "b_kv": b_kv, "W_q": W_q,
        "b_q": b_q, "W_o": W_o,
    }
    b_kv = np.asarray(b_kv, np.float32)
    b_o = np.asarray(b_o, np.float32)
    W_o_np = np.asarray(W_o, np.float32)
    # v-bias folds into the output bias: out += W_o @ b_v + b_o
    b_v_vec = np.concatenate(
        [b_kv[h * 2 * DH + DH:(h + 1) * 2 * DH] for h in range(H)], 0)
    b_eff = b_o + W_o_np @ b_v_vec
    masked = bool(np.any(np.asarray(cross_mask)))
    nc = _get_built(masked)
    in_maps = _shard_inputs(inputs, masked)

    res = run_bass_kernel_spmd(nc, in_maps, core_ids=list(range(N_CORES)))
    outs = [res.results[c]["out"] for c in range(N_CORES)]
    full = np.stack([outs[2 * b] + outs[2 * b + 1] for b in range(B)], 0)
    return (full + b_eff[None, None, :]).astype(np.float32)


# revision 12
# speedup vs baseline: 1.0635x; 1.0051x over previous
"""Multi-head cross-attention on 8 Trainium2 NeuronCores.

Problem shapes (hardcoded): B=4, Ld=1024, Le=2048, d_model=1024, 8 heads x 128.
Sharding: core c handles batch b=c//2 and head-group g=c%2 (4 heads each).
Each core computes q/k/v projections for its heads, attention, and a partial
output projection over its heads' value dims; the host sums the two partial
outputs per batch and adds the (bias-folded) output bias.

Key scheduling/engine decisions:
- Projection inputs (enc, x, wq, wk, wv) ship as bf16 in chunk-major host
  layout ([128, chunks*cols]), so each tensor is one large contiguous DMA;
  only the cold-start K-proj inputs stream per chunk.
- Projections run d-major across all 8 PSUM banks; bank copy-backs alternate
  DVE/Act so the next phase's bank-0 chain never waits.
- Softmax denominators never touch the PE: exp'd score chunks (bf16)
  accumulate on DVE (2x mode), then one gpsimd partition_all_reduce per
  (q-half, head) produces the broadcast denominator, reciprocal on DVE.
- exp processes two key-chunks per Act instruction ([128,1024] spanning two
  PSUM banks) to keep Act throughput above the PE's score+AV rate.
- b_v is folded into the host-side output bias (b_eff = b_o + W_o @ b_v),
  removing all bias work from the attention loop.
- Attention is software-pipelined: one filler matmul per inner step keeps
  the PE busy while Act computes exp. Fillers for q-half 0 are the second
  half of the Q projection; fillers for q-half 1 are q-half 0's output
  projection. The PE is warmed with dummy matmuls on a memset tile so the
  p-state ramp completes before real work arrives.
"""

import math
import sys

import numpy as np

for _p in ("/opt/trn_rl_repo", "/root/.axon_site/_ro/trn_rl_repo"):
    if _p not in sys.path:
        sys.path.append(_p)

B = 4
LQ = 1024
LK = 2048
D = 1024
H = 8
DH = 128
P = 128
HPC = 4          # heads per core
OQ = HPC * DH    # 512 projected dims per core
NQ = 512         # matmul moving free dim
KC = D // P      # 8 contraction chunks for projections
LKC = LK // P    # 16 key chunks
HLK = LK // 2    # 1024, one lk-half of the encoder
N_CORES = 8

_BUILT = {}


def _build(masked):
    import concourse.bass as bass  # noqa: F401
    import concourse.tile as tile
    import concourse.mybir as mybir
    from concourse import bacc
    from concourse import bass_isa

    f32 = mybir.dt.float32
    f32r = mybir.dt.float32r
    bf16 = mybir.dt.bfloat16
    Exp = mybir.ActivationFunctionType.Exp

    nc = bacc.Bacc("TRN2", target_bir_lowering=False, debug=False,
                   num_devices=N_CORES)

    # chunk-major DRAM layouts: [...][p, d*cols + c] = chunk d, row p, col c
    xr = nc.dram_tensor("xr", [P, 2 * KC * NQ], bf16, kind="ExternalInput").ap()
    e0r = nc.dram_tensor("e0r", [P, KC * HLK], bf16, kind="ExternalInput").ap()
    e1r = nc.dram_tensor("e1r", [P, KC * HLK], bf16, kind="ExternalInput").ap()
    wqr = nc.dram_tensor("wqr", [P, KC * OQ], bf16, kind="ExternalInput").ap()
    wkr = nc.dram_tensor("wkr", [P, KC * OQ], bf16, kind="ExternalInput").ap()
    wvr = nc.dram_tensor("wvr", [P, KC * OQ], bf16, kind="ExternalInput").ap()
    wor = nc.dram_tensor("wor", [P, HPC * D], f32r, kind="ExternalInput").ap()
    bq_d = nc.dram_tensor("bq", [P, HPC], f32, kind="ExternalInput").ap()
    bk_d = nc.dram_tensor("bk", [P, HPC], f32, kind="ExternalInput").ap()
    if masked:
        maskT = nc.dram_tensor("maskT", [LK, LQ], f32, kind="ExternalInput").ap()
    out_d = nc.dram_tensor("out", [LQ, D], bf16, kind="ExternalOutput").ap()

    with tile.TileContext(nc) as tc:
        with tc.tile_pool(name="persist", bufs=1) as persist:
            kT = [persist.tile([P, LK], f32r, name=f"kT{h}") for h in range(HPC)]
            qT = [persist.tile([P, LQ], f32r, name=f"qT{h}") for h in range(HPC)]
            vch = [persist.tile([P, OQ], bf16, name=f"v{j}") for j in range(LKC)]
            wq_all = persist.tile([P, KC * OQ], bf16, name="wq")
            wo_all = persist.tile([P, HPC * D], f32r, name="wo")
            x_all = persist.tile([P, 2 * KC * NQ], bf16, name="x")
            bq_sb = persist.tile([P, HPC], f32, name="bq")
            bk_sb = persist.tile([P, HPC], f32, name="bk")
            warm = persist.tile([P, 256], bf16, name="warm")

            wqc = [wq_all[:, d * OQ:(d + 1) * OQ] for d in range(KC)]
            woch = [wo_all[:, h * D:(h + 1) * D] for h in range(HPC)]
            xg0 = [x_all[:, d * NQ:(d + 1) * NQ] for d in range(KC)]
            xg1 = [x_all[:, (KC + d) * NQ:(KC + d + 1) * NQ] for d in range(KC)]

            with (
                tc.tile_pool(name="acc", bufs=1, space="PSUM") as accp,
                tc.tile_pool(name="wk", bufs=1) as wkp,
                tc.tile_pool(name="wv", bufs=1) as wvp,
                tc.tile_pool(name="e0p", bufs=1) as e0p,
                tc.tile_pool(name="e1p", bufs=1) as e1p,
            ):
                banks = [accp.tile([P, NQ], f32, name=f"bank{t}")
                         for t in range(8)]
                wk_all = wkp.tile([P, KC * OQ], bf16, name="wk")
                wv_all = wvp.tile([P, KC * OQ], bf16, name="wv")
                e0_all = e0p.tile([P, KC * HLK], bf16, name="e0")
                e1_all = e1p.tile([P, KC * HLK], bf16, name="e1")
                wkc = [wk_all[:, d * OQ:(d + 1) * OQ] for d in range(KC)]
                wvc = [wv_all[:, d * OQ:(d + 1) * OQ] for d in range(KC)]
                e0 = [e0_all[:, d * HLK:(d + 1) * HLK] for d in range(KC)]
                e1 = [e1_all[:, d * HLK:(d + 1) * HLK] for d in range(KC)]

                # --- PE warm-up: memset a tile (no DMA) and run dummy
                # matmuls so the p-state ramp happens before real work.
                nc.vector.memset(warm[:], 1.0)
                for _ in range(14):
                    nc.tensor.matmul(banks[0][0:16, 0:256], warm[:, 0:16],
                                     warm[:], start=True, stop=True)

                # --- DMA program: cold-start K inputs per chunk, the rest
                # as single contiguous transfers in need-order.
                for d in range(KC):
                    nc.sync.dma_start(wkc[d], wkr[:, d * OQ:(d + 1) * OQ])
                    nc.sync.dma_start(e0[d], e0r[:, d * HLK:(d + 1) * HLK])
                nc.sync.dma_start(bq_sb[:], bq_d[:])
                nc.sync.dma_start(bk_sb[:], bk_d[:])
                nc.sync.dma_start(wv_all[:], wvr[:])
                nc.sync.dma_start(e1_all[:], e1r[:])
                nc.sync.dma_start(wq_all[:], wqr[:])
                nc.sync.dma_start(x_all[:], xr[:])
                nc.sync.dma_start(wo_all[:], wor[:])

                def kproj_half(e, lh):
                    for d in range(KC):
                        for g in range(2):
                            for h in range(HPC):
                                nc.tensor.matmul(
                                    banks[g * 4 + h][:],
                                    wkc[d][:, h * DH:(h + 1) * DH],
                                    e[d][:, g * NQ:(g + 1) * NQ],
                                    start=(d == 0), stop=(d == KC - 1))
                    for i in range(8):
                        g, h = i // 4, i % 4
                        dst = kT[h][:, lh * HLK + g * NQ:lh * HLK + (g + 1) * NQ]
                        if i % 2 == 0:
                            nc.vector.tensor_scalar_add(
                                dst, banks[i][:], bk_sb[:, h:h + 1])
                        else:
                            nc.scalar.add(dst, banks[i][:], bk_sb[:, h:h + 1])

                def vproj_half(e, lh):
                    for d in range(KC):
                        for j8 in range(8):
                            nc.tensor.matmul(
                                banks[j8][:],
                                e[d][:, j8 * P:(j8 + 1) * P],
                                wvc[d],
                                start=(d == 0), stop=(d == KC - 1))
                    for j8 in range(8):
                        dst = vch[lh * 8 + j8][:]
                        if j8 % 2 == 0:
                            nc.vector.tensor_copy(dst, banks[j8][:])
                        else:
                            nc.scalar.copy(dst, banks[j8][:])

                kproj_half(e0, 0)
                vproj_half(e0, 0)
                kproj_half(e1, 1)
                vproj_half(e1, 1)

                # Q proj, query half 0, in two 2-head subphases so the
                # head-0/1 copy-backs finish before attention needs qT.
                for sub in range(2):
                    hs = (0, 1) if sub == 0 else (2, 3)
                    for d in range(KC):
                        for h in hs:
                            nc.tensor.matmul(
                                banks[h][:],
                                wqc[d][:, h * DH:(h + 1) * DH],
                                xg0[d],
                                start=(d == 0), stop=(d == KC - 1))
                    for h in hs:
                        nc.scalar.add(qT[h][:, 0:NQ], banks[h][:],
                                      bq_sb[:, h:h + 1])

            # ---- Attention, software-pipelined with filler matmuls.
            with (
                tc.tile_pool(name="pTp", bufs=12) as pTp,
                tc.tile_pool(name="dnp", bufs=2) as dnp,
                tc.tile_pool(name="maskp", bufs=16 if masked else 1) as maskp,
                tc.tile_pool(name="osb", bufs=4) as osb,
                tc.tile_pool(name="att", bufs=1) as attp,
                tc.tile_pool(name="pss", bufs=2, space="PSUM") as pss,
                tc.tile_pool(name="psa", bufs=2, space="PSUM") as psa,
                tc.tile_pool(name="psx", bufs=2, space="PSUM") as psx,
            ):
                valsT = [attp.tile([P, LQ], f32r, name=f"valsT{h}")
                         for h in range(HPC)]
                partial = [attp.tile([P, NQ], f32, name=f"prt{c}")
                           for c in range(8)]

                def attn_q2(q2, slot_fillers):
                    """h-loops for one query half. slot_fillers: 32 lists of
                    callables (slot = h*8+g), each emitting one PE matmul
                    (+ its own non-PE follow-ups)."""

                    def fill(slot):
                        for f in slot_fillers[slot]:
                            f()

                    if masked:
                        mch = []
                        for j in range(LKC):
                            mt = maskp.tile([P, NQ], f32, name=f"m{j}")
                            nc.sync.dma_start(
                                mt[:], maskT[j * P:(j + 1) * P,
                                             q2 * NQ:(q2 + 1) * NQ])
                            mch.append(mt)

                    for h in range(HPC):
                        qs = qT[h][:, q2 * NQ:(q2 + 1) * NQ]
                        ps_v = psa.tile([P, NQ], f32, name="ps_v")
                        pT = [None] * 8
                        acc = None

                        def spair(g):
                            t = pss.tile([P, 2 * NQ], f32, name="ps_s")
                            for jj in range(2):
                                j = 2 * g + jj
                                nc.tensor.matmul(
                                    t[:, jj * NQ:(jj + 1) * NQ],
                                    kT[h][:, j * P:(j + 1) * P],
                                    qs, start=True, stop=True)
                            return t

                        def do_exp(g, t):
                            if masked:
                                for jj in range(2):
                                    j = 2 * g + jj
                                    nc.vector.tensor_add(
                                        t[:, jj * NQ:(jj + 1) * NQ],
                                        t[:, jj * NQ:(jj + 1) * NQ],
                                        mch[j][:])
                            p = pTp.tile([P, 2 * NQ], bf16, name="pT")
                            nc.scalar.activation(p[:], t[:], Exp)
                            pT[g] = p

                        def avpair(g):
                            for jj in range(2):
                                j = 2 * g + jj
                                nc.tensor.matmul(
                                    ps_v[:],
                                    vch[j][:, h * DH:(h + 1) * DH],
                                    pT[g][:, jj * NQ:(jj + 1) * NQ],
                                    start=(j == 0), stop=(j == LKC - 1))

                        accp_ = None
                        st = [spair(0)]
                        st.append(spair(1))
                        for g in range(8):
                            do_exp(g, st[g])
                            if g + 2 < 8:
                                st.append(spair(g + 2))
                            fill(h * 8 + g)
                            avpair(g)
                            # denominator accumulation: DVE takes pairs
                            # 0,4..7 (bf16 2x), the idle Pool engine takes
                            # pairs 1..3; merged before the all-reduce.
                            p = pT[g]
                            dve_only = (q2 == 1 and h == 3)
                            if g == 0:
                                acc = dnp.tile([P, NQ], bf16, name="dacc")
                                nc.vector.tensor_add(
                                    acc[:], p[:, 0:NQ], p[:, NQ:2 * NQ])
                            elif dve_only:
                                nc.vector.tensor_add(
                                    acc[:], acc[:], p[:, 0:NQ])
                                nc.vector.tensor_add(
                                    acc[:], acc[:], p[:, NQ:2 * NQ])
                            elif g == 1:
                                accp_ = dnp.tile([P, NQ], bf16, name="daccp")
                                nc.gpsimd.tensor_add(
                                    accp_[:], p[:, 0:NQ], p[:, NQ:2 * NQ])
                            elif g <= 3:
                                nc.gpsimd.tensor_add(
                                    accp_[:], accp_[:], p[:, 0:NQ])
                                nc.gpsimd.tensor_add(
                                    accp_[:], accp_[:], p[:, NQ:2 * NQ])
                            else:
                                nc.vector.tensor_add(
                                    acc[:], acc[:], p[:, 0:NQ])
                                nc.vector.tensor_add(
                                    acc[:], acc[:], p[:, NQ:2 * NQ])
                        if accp_ is not None:
                            nc.vector.tensor_add(acc[:], acc[:], accp_[:])
                        dbc = dnp.tile([P, NQ], f32, name="dbc")
                        nc.gpsimd.partition_all_reduce(
                            dbc[:], acc[:], channels=P,
                            reduce_op=bass_isa.ReduceOp.add)
                        rr = dnp.tile([P, NQ], f32, name="rr")
                        nc.vector.reciprocal(rr[:], dbc[:])
                        nc.vector.tensor_mul(
                            valsT[h][:, q2 * NQ:(q2 + 1) * NQ],
                            ps_v[:], rr[:])

                ostate = {}

                def out_dst(q2, c):
                    lqc = q2 * 4 + c // 2
                    o2 = c % 2
                    return out_d[lqc * P:(lqc + 1) * P,
                                 o2 * NQ:(o2 + 1) * NQ]

                def op_part(q2, c, hh, start, stop):
                    """One head-part of output-projection chain (q2, c)."""
                    lqc = q2 * 4 + c // 2
                    o2 = c % 2
                    key = (q2, c)
                    if start:
                        ostate[key] = psx.tile([P, NQ], f32, name="px")
                    nc.tensor.matmul(
                        ostate[key][:],
                        valsT[hh][:, lqc * P:(lqc + 1) * P],
                        woch[hh][:, o2 * NQ:(o2 + 1) * NQ],
                        start=start, stop=stop)

                def op_store(q2, c, split=False, on_act=False):
                    po = ostate[(q2, c)]
                    dst = out_dst(q2, c)
                    if split:
                        for half in range(2):
                            sl = slice(half * (NQ // 2), (half + 1) * (NQ // 2))
                            ot = osb.tile([P, NQ // 2], bf16, name="ot2")
                            nc.vector.tensor_copy(ot[:], po[:, sl])
                            nc.sync.dma_start(dst[:, sl], ot[:])
                    else:
                        ot = osb.tile([P, NQ], bf16, name="ot")
                        if on_act:
                            nc.scalar.copy(ot[:], po[:])
                        else:
                            nc.vector.tensor_copy(ot[:], po[:])
                        nc.sync.dma_start(dst, ot[:])

                def b_filler(c, hh):
                    """op0 chain part; full 4-head chain + store."""
                    def f():
                        op_part(0, c, hh, start=(hh == 0), stop=(hh == 3))
                        if hh == 3:
                            op_store(0, c)
                    return f

                def a_filler(hh, d):
                    """q-projection (half 1) d-chain part for head hh."""
                    def f():
                        key = ("q", hh)
                        if d == 0:
                            ostate[key] = psx.tile([P, NQ], f32, name="px")
                        nc.tensor.matmul(
                            ostate[key][:],
                            wqc[d][:, hh * DH:(hh + 1) * DH],
                            xg1[d],
                            start=(d == 0), stop=(d == KC - 1))
                        if d == KC - 1:
                            nc.vector.tensor_scalar_add(
                                qT[hh][:, NQ:2 * NQ], ostate[key][:],
                                bq_sb[:, hh:hh + 1])
                    return f

                # q-half 0: fillers are the 4 q-proj(g1) head chains,
                # front-loaded so the exp pipeline-fill gap is covered.
                parts0 = [a_filler(s // 8, s % 8) for s in range(32)]
                slots0 = [[] for _ in range(32)]
                slots0[0] = [parts0[0], parts0[1]]
                slots0[1] = [parts0[2], parts0[3]]
                for i, pf in enumerate(parts0[4:]):
                    slots0[2 + i].append(pf)
                attn_q2(0, slots0)

                # q-half 1: op0 chains B0-B4 fill loops h0-h2; B5-B7 are
                # reserved to cover the final norm chain post-loop.
                slots1 = [[] for _ in range(32)]
                for c in range(5):
                    base = c * 4
                    for hh in range(4):
                        slots1[base + hh].append(b_filler(c, hh))
                attn_q2(1, slots1)

                # Post-loop: reserves bridge the last norm wait, then the
                # final out-projection drains with copies split across
                # Act (idle once exps end) and DVE.
                for c in (5, 6, 7):
                    for hh in range(4):
                        op_part(0, c, hh, start=(hh == 0), stop=(hh == 3))
                    op_store(0, c, on_act=True)
                for c in range(8):
                    for hh in range(4):
                        op_part(1, c, hh, start=(hh == 0), stop=(hh == 3))
                    if c == 7:
                        op_store(1, c, split=True)
                    else:
                        op_store(1, c, on_act=(c % 2 == 0))

    nc.compile()
    return nc


def _get_built(masked):
    if masked not in _BUILT:
        _BUILT[masked] = _build(masked)
    return _BUILT[masked]


def _chunk_major(a, n_chunks):
    """[n_chunks*P, C] -> [P, n_chunks*C] with [p, d*C+c] = a[d*P+p, c]."""
    C = a.shape[1]
    return np.ascontiguousarray(
        a.reshape(n_chunks, P, C).transpose(1, 0, 2).reshape(P, n_chunks * C))


def _shard_inputs(inputs, masked):
    import ml_dtypes

    bf16 = ml_dtypes.bfloat16

    x = np.asarray(inputs["mhca_input"], np.float32)
    enc = np.asarray(inputs["encoder_output"], np.float32)
    mask = np.asarray(inputs["cross_mask"], np.float32)
    W_kv = np.asarray(inputs["W_kv"], np.float32)
    b_kv = np.asarray(inputs["b_kv"], np.float32)
    W_q = np.asarray(inputs["W_q"], np.float32)
    b_q = np.asarray(inputs["b_q"], np.float32)
    W_o = np.asarray(inputs["W_o"], np.float32)

    scale = 1.0 / math.sqrt(DH)
    in_maps = []
    for c in range(N_CORES):
        b = c // 2
        g = c % 2
        heads = list(range(g * HPC, (g + 1) * HPC))
        sl = slice(g * OQ, (g + 1) * OQ)
        k_rows = np.concatenate(
            [W_kv[h * 2 * DH:h * 2 * DH + DH] for h in heads], 0)
        v_rows = np.concatenate(
            [W_kv[h * 2 * DH + DH:(h + 1) * 2 * DH] for h in heads], 0)
        xT = x[b].T                                   # [D, LQ]
        encT = enc[b].T                               # [D, LK]
        m = {
            "xr": _chunk_major(
                np.concatenate([xT[:, :NQ], xT[:, NQ:]], 0), 2 * KC
            ).astype(bf16),
            "e0r": _chunk_major(encT[:, :HLK], KC).astype(bf16),
            "e1r": _chunk_major(encT[:, HLK:], KC).astype(bf16),
            "wqr": _chunk_major((W_q[sl] * scale).T, KC).astype(bf16),
            "wkr": _chunk_major(k_rows.T, KC).astype(bf16),
            "wvr": _chunk_major(v_rows.T, KC).astype(bf16),
            "wor": _chunk_major(np.ascontiguousarray(W_o[:, sl].T), HPC),
            "bq": np.ascontiguousarray((b_q[sl] * scale).reshape(HPC, DH).T),
            "bk": np.ascontiguousarray(
                np.stack([b_kv[h * 2 * DH:h * 2 * DH + DH] for h in heads], 1)),
        }
        if masked:
            m["maskT"] = np.ascontiguousarray(mask[b].T)
        in_maps.append(m)
    return in_maps


def kernel(mhca_input, encoder_output, cross_mask, W_kv, b_kv, W_q, b_q, W_o,
           b_o):
    from concourse.bass_utils import run_bass_kernel_spmd

    inputs = {
        "mhca_input": mhca_input, "encoder_output": encoder_output,
        "cross_mask": cross_mask, "W_kv": W_kv, "b_kv": b_kv, "W_q": W_q,
        "b_q": b_q, "W_o": W_o,
    }
    b_kv = np.asarray(b_kv, np.float32)
    b_o = np.asarray(b_o, np.float32)
    W_o_np = np.asarray(W_o, np.float32)
    # v-bias folds into the output bias: out += W_o @ b_v + b_o
    b_v_vec = np.concatenate(
        [b_kv[h * 2 * DH + DH:(h + 1) * 2 * DH] for h in range(H)], 0)
    b_eff = b_o + W_o_np @ b_v_vec
    masked = bool(np.any(np.asarray(cross_mask)))
    nc = _get_built(masked)
    in_maps = _shard_inputs(inputs, masked)

    res = run_bass_kernel_spmd(nc, in_maps, core_ids=list(range(N_CORES)))
    outs = [np.asarray(res.results[c]["out"], np.float32)
            for c in range(N_CORES)]
    full = np.stack([outs[2 * b] + outs[2 * b + 1] for b in range(B)], 0)
    return (full + b_eff[None, None, :]).astype(np.float32)
